# revision 4
# baseline (speedup 1.0000x reference)
"""Trainium2 Bass kernel v3 for nn_Net_20512763805724 (3-layer binary SLP).

Math (bit-model validated, relmax ~2.7e-3 vs fp32 reference):
  Layer with input p [B,L], binarized weight wb [O,L] (+-1):
    out = (C + s^2)/L^2,  s = 2 p@wb.T - c,  c[o] = sum_f wb[o,f],
    C[b] = 4 sum_f p(1-p) = L - 4 sum_f u^2   with u = p - 0.5
  In u-space s = 2 u@wb.T (the -c/2 comes free from the u offset), so per
  layer only a matmul on u, an elementwise square u^2, and a (-ones)
  matmul reduction of u^2 are needed.  The affine constants cascade into
  the next layer's ACT biases (kappa cascade); the final additive 1/L3
  lands in psB3 via a K=1 const-matmul.

Schedule: 8 cores x 8192 rows, 16 tiles of 512 processed in PAIRS.  The
pair's layer-1 matmuls are issued adjacently with tile_position column
packing (0,0)/(0,64) -> the PE runs both concurrently (~135 ns/MM for
N=512 instead of ~216).  Squares split: ACT takes chunks 0-2 (Square,
free bias), DVE tensor_tensor takes chunks 3-5 (one big strided op).
Layer-2 matmuls are deferred to the quad boundary so their two
block-diagonal MMs also pair concurrently.
"""

import sys

if "/opt/trn_rl_repo" not in sys.path:
    sys.path.insert(0, "/opt/trn_rl_repo")

import ml_dtypes
import numpy as np

BF16 = ml_dtypes.bfloat16

B = 65536
IN_DIM = 768
NCORES = 8
BC = B // NCORES            # 8192 rows per core
TILE = 512
NT = BC // TILE             # 16 tiles per core
NPAIR = NT // 2
NCH = 6
L1, O1 = 768, 64
L2, O2 = 64, 32
L3, O3 = 32, 4
S1, S2, S3 = 4.0 / L1**2, 4.0 / L2**2, 4.0 / L3**2
K1 = 1.0 / L1 - 0.5
K2 = 1.0 / L2 - 0.5
KF_W = 8.0                  # (1/L3)/S3, exact in bf16
# gp variant: sq ops compute pure u^2; the 2*kappa*u cross-terms come from
# correction matmuls (lhsT = -2*kappa * block-diag ones) and the kappa^2
# residues cascade into bias3/KF.
K2G = 1.0 / L2 - 0.5 - L2 * S2 * K1 * K1
KFG_W = float(np.float32(BF16((1.0 / L3 - L3 * S3 * K2G * K2G) / S3)))
NACT = 2                    # chunks squared on ACT; rest on DVE

_CACHE = {}


def _host_prep(x, w1, w2, w3):
    """Host-side layout/cast prep (per-core input maps)."""
    x = np.asarray(x, dtype=np.float32)
    u = x - 0.5
    # [core, pair, tile-in-pair, batch, chunk, p] -> [core, pair, p, t, c, b]
    us = u.reshape(NCORES, NPAIR, 2, TILE, NCH, 128)
    # -> [core, pair, p, chunk, tile-in-pair, batch]
    ub = np.ascontiguousarray(us.transpose(0, 1, 5, 4, 2, 3)).astype(BF16)

    def sign(w):
        return np.where(np.asarray(w, np.float32) >= 0, 1.0, -1.0).astype(
            np.float32
        )

    wb1, wb2, wb3 = sign(w1), sign(w2), sign(w3)
    c2 = wb2.sum(1)
    c3 = wb3.sum(1)
    # w1p [128, 6, 64]: (p, c, m) = wb1[m, 128*c + p]
    w1p = np.ascontiguousarray(wb1.T.reshape(NCH, 128, O1).transpose(1, 0, 2))
    # layer-2 block-diag [128, 64]: rows 64*h+l, cols 32*h+o = wb2[o, l]
    w2bd = np.zeros((128, 2 * O2), np.float32)
    w2bdn = np.zeros((128, 2 * O2), np.float32)
    for h in range(2):
        w2bd[64 * h : 64 * h + 64, 32 * h : 32 * h + 32] = wb2.T
        w2bdn[64 * h : 64 * h + 64, 32 * h : 32 * h + 32] = -1.0
    # layer-3 block-diag [128, 16]: rows 32*j+l, cols 4*j+o = wb3[o, l]
    w3bd = np.zeros((128, 4 * O3), np.float32)
    w3bdn = np.zeros((128, 4 * O3), np.float32)
    for j in range(4):
        w3bd[32 * j : 32 * j + 32, 4 * j : 4 * j + 4] = wb3.T
        w3bdn[32 * j : 32 * j + 32, 4 * j : 4 * j + 4] = -1.0
    # psA2 rows are [64*pr2 + 32*h + o] -> bias pattern repeats every 32
    bias2 = np.tile((2.0 * K1 / L2 * c2)[:, None], (4, 1)).astype(np.float32)
    # psA3 rows are [4*tq + o], tq = 0..3 (16 rows used)
    bias3 = np.zeros((128, 1), np.float32)
    bias3[0:16, 0] = np.tile(2.0 * K2 / L3 * c3, 4)
    bias3g = np.zeros((128, 1), np.float32)
    bias3g[0:16, 0] = np.tile(2.0 * K2G / L3 * c3, 4)
    wmap = {
        "w1p": w1p,
        "w2bd": w2bd,
        "w2bdn": w2bdn,
        "w3bd": w3bd,
        "w3bdn": w3bdn,
        "bias2": bias2,
        "bias3": bias3,
        "bias3g": bias3g,
        "w2bdk": (-2.0 * K1) * w2bdn * -1.0,
        "w3bdk": (-2.0 * K2G) * w3bdn * -1.0,
    }
    return [{"ub": ub[i], **wmap} for i in range(NCORES)]


def kernel(x, w1, w2, w3):
    from concourse.bass_utils import run_bass_kernel_spmd

    nc = _get_nc()
    in_maps = _host_prep(x, w1, w2, w3)
    res = run_bass_kernel_spmd(nc, in_maps, core_ids=list(range(NCORES)))
    return np.concatenate(
        [res.results[i]["outt"].T for i in range(NCORES)], axis=0
    ).astype(np.float32)


def _get_nc(reps=1, **opt):
    key = ("nc", reps, tuple(sorted(opt.items())))
    if key not in _CACHE:
        _CACHE[key] = _build(reps, **opt)
    return _CACHE[key]


def _build(reps=1, mode="full", nact=NACT, xbufs=6, sbufs=4, qbufs=4,
           interleave=False, tt6=False, gp=False, sqfirst=False,
           outsync=False, pa3b1=False, dmahalf=False, sqdve=True):
    import contextlib

    import concourse.bacc as bacc
    import concourse.mybir as mybir
    import concourse.tile as tile

    f32 = mybir.dt.float32
    bf16 = mybir.dt.bfloat16
    AOP = mybir.AluOpType
    Square = mybir.ActivationFunctionType.Square

    nc = bacc.Bacc(None, target_bir_lowering=False)

    ubd = nc.declare_dram_parameter("ub", [NPAIR, 128, NCH, 2, TILE], bf16,
                                    isOutput=False)
    w1pd = nc.declare_dram_parameter("w1p", [128, NCH, O1], f32, isOutput=False)
    w2bdd = nc.declare_dram_parameter("w2bd", [128, 64], f32, isOutput=False)
    w2bdnd = nc.declare_dram_parameter("w2bdn", [128, 64], f32, isOutput=False)
    w3bdd = nc.declare_dram_parameter("w3bd", [128, 16], f32, isOutput=False)
    w3bdnd = nc.declare_dram_parameter("w3bdn", [128, 16], f32, isOutput=False)
    bias2d = nc.declare_dram_parameter("bias2", [128, 1], f32, isOutput=False)
    bias3d = nc.declare_dram_parameter("bias3", [128, 1], f32, isOutput=False)
    bias3gd = nc.declare_dram_parameter("bias3g", [128, 1], f32, isOutput=False)
    w2bdkd = nc.declare_dram_parameter("w2bdk", [128, 64], f32, isOutput=False)
    w3bdkd = nc.declare_dram_parameter("w3bdk", [128, 16], f32, isOutput=False)
    outt = nc.declare_dram_parameter("outt", [O3, BC], f32, isOutput=True)

    with tile.TileContext(nc) as tc:
        with (
            tc.tile_pool(name="const", bufs=1) as cpool,
            tc.tile_pool(name="xp", bufs=xbufs) as xpool,
            tc.tile_pool(name="sq", bufs=qbufs) as qpool,
            tc.tile_pool(name="sb", bufs=sbufs) as spool,
            tc.tile_pool(name="pA1", bufs=3 if pa3b1 else 2,
                         space="PSUM") as pA1,
            tc.tile_pool(name="pB1", bufs=1 if pa3b1 else 2,
                         space="PSUM") as pB1,
            tc.tile_pool(name="pL2", bufs=1, space="PSUM") as pL2,
            tc.tile_pool(name="pL3", bufs=1, space="PSUM") as pL3,
        ):
            # ---------------- weight prep (one-time) ----------------
            w1raw = cpool.tile([128, NCH, O1], f32)
            nc.sync.dma_start(out=w1raw, in_=w1pd[:])
            w1b = cpool.tile([128, NCH, O1], bf16)
            nc.vector.tensor_scalar(w1b, w1raw, 1.0, None, AOP.mult)
            onesb = cpool.tile([128, O1], bf16)
            nc.vector.memset(onesb, -1.0)

            w2raw = cpool.tile([128, 64], f32)
            nc.sync.dma_start(out=w2raw, in_=w2bdd[:])
            w2bd = cpool.tile([128, 64], bf16)
            nc.vector.tensor_scalar(w2bd, w2raw, 1.0, None, AOP.mult)
            w2nraw = cpool.tile([128, 64], f32)
            nc.sync.dma_start(out=w2nraw, in_=w2bdnd[:])
            w2bdn = cpool.tile([128, 64], bf16)
            nc.vector.tensor_scalar(w2bdn, w2nraw, 1.0, None, AOP.mult)

            w3raw = cpool.tile([128, 16], f32)
            nc.sync.dma_start(out=w3raw, in_=w3bdd[:])
            w3bd = cpool.tile([128, 16], bf16)
            nc.vector.tensor_scalar(w3bd, w3raw, 1.0, None, AOP.mult)
            w3nraw = cpool.tile([128, 16], f32)
            nc.sync.dma_start(out=w3nraw, in_=w3bdnd[:])
            w3bdn = cpool.tile([128, 16], bf16)
            nc.vector.tensor_scalar(w3bdn, w3nraw, 1.0, None, AOP.mult)

            bias2 = cpool.tile([128, 1], f32)
            nc.sync.dma_start(out=bias2, in_=bias2d[:])
            bias3 = cpool.tile([128, 1], f32)
            if gp or sqdve:
                nc.sync.dma_start(out=bias3, in_=bias3gd[:])
            else:
                nc.sync.dma_start(out=bias3, in_=bias3d[:])
            w2kraw = cpool.tile([128, 64], f32)
            nc.sync.dma_start(out=w2kraw, in_=w2bdkd[:])
            w2bdk = cpool.tile([128, 64], bf16)
            nc.vector.tensor_scalar(w2bdk, w2kraw, 1.0, None, AOP.mult)
            w3kraw = cpool.tile([128, 16], f32)
            nc.sync.dma_start(out=w3kraw, in_=w3bdkd[:])
            w3bdk = cpool.tile([128, 16], bf16)
            nc.vector.tensor_scalar(w3bdk, w3kraw, 1.0, None, AOP.mult)
            zbias = cpool.tile([128, 1], f32)
            nc.vector.memset(zbias, 0.0)
            sqb2 = cpool.tile([128, 1], f32)
            nc.vector.memset(sqb2, K1)
            sqb3 = cpool.tile([128, 1], f32)
            nc.vector.memset(sqb3, K2)
            kf_w = cpool.tile([1, 16], bf16)
            nc.vector.memset(kf_w, KFG_W if (gp or sqdve) else KF_W)
            ones_row = cpool.tile([1, TILE], bf16)
            nc.vector.memset(ones_row, 1.0)

            # ---------------- main loop (pairs of 512-row tiles) -----
            loop_cm = (
                tc.For_i(0, reps, 1) if reps > 1 else contextlib.nullcontext()
            )
            with loop_cm:
              u2ps = [None, None]
              sq2s = [None, None]
              psA2 = psB2 = None
              for pr in range(NPAIR):
                pr2 = pr % 2
                qj = pr % 2          # pair index within quad

                ut = xpool.tile([128, NCH, 2, TILE], bf16, tag="u",
                                name=f"u_{pr}")
                if dmahalf:
                    nc.sync.dma_start(out=ut[:, 0:3, :, :],
                                      in_=ubd[pr, :, 0:3, :, :])
                    nc.sync.dma_start(out=ut[:, 3:NCH, :, :],
                                      in_=ubd[pr, :, 3:NCH, :, :])
                else:
                    nc.sync.dma_start(out=ut, in_=ubd[pr])

                psA1 = pA1.tile([128, TILE], f32, tag="A1", name=f"A1_{pr}")
                psB1 = pB1.tile([128, TILE], f32, tag="B1", name=f"B1_{pr}")

                def emit_sq():
                    sqa = qpool.tile([128, nact, 2, TILE], bf16, tag="sqa",
                                     name=f"sqa_{pr}")
                    for c in range(nact):
                        nc.scalar.activation(
                            sqa[:, c, :, :], ut[:, c, :, :], Square,
                            bias=zbias, scale=1.0,
                        )
                    sqb = qpool.tile([128, NCH - nact, 2, TILE], bf16,
                                     tag="sqb", name=f"sqb_{pr}")
                    nc.vector.tensor_tensor(
                        sqb, ut[:, nact:, :, :], ut[:, nact:, :, :], AOP.mult
                    )
                    return sqa, sqb

                if sqfirst and mode == "full":
                    sqa, sqb = emit_sq()

                # layer-1 matmuls: adjacent col-tiled duos (concurrent)
                a1order = (
                    [(c, tl) for c in range(NCH) for tl in range(2)]
                    if interleave else
                    [(c, tl) for tl in range(2) for c in range(NCH)]
                )
                for c, tl in a1order:
                    nc.tensor.matmul(
                        psA1[O1 * tl : O1 * (tl + 1), :],
                        w1b[:, c, :],
                        ut[:, c, tl, :],
                        start=(c == 0),
                        stop=(c == NCH - 1),
                        tile_position=(0, O1 * tl),
                        skip_group_check=True,
                    )
                if mode == "a1":
                    t2 = spool.tile([128, TILE], f32, tag="t2",
                                    name=f"t2_{pr}")
                    nc.scalar.activation(
                        t2, psA1, Square, bias=zbias, scale=2.0 / L1
                    )
                    qd2 = pr // 2
                    nc.scalar.dma_start(
                        out=outt[:, qd2 * TILE : qd2 * TILE + TILE],
                        in_=t2[0:4, :],
                    )
                    continue

                # squares: ACT chunks 0..NACT-1, DVE TT chunks NACT..5
                if not (sqfirst and mode == "full"):
                    sqa, sqb = emit_sq()

                # C-term: ones-matmuls, adjacent col-tiled duos
                for c in range(NCH):
                    src = (
                        sqa[:, c, :, :] if c < nact
                        else sqb[:, c - nact, :, :]
                    )
                    for tl in range(2):
                        nc.tensor.matmul(
                            psB1[O1 * tl : O1 * (tl + 1), :],
                            onesb,
                            src[:, tl, :],
                            start=(c == 0),
                            stop=(c == NCH - 1),
                            tile_position=(0, O1 * tl),
                            skip_group_check=True,
                        )

                # layer-1 combine for the pair
                t2 = spool.tile([128, TILE], f32, tag="t2", name=f"t2_{pr}")
                nc.scalar.activation(
                    t2, psA1, Square, bias=zbias, scale=2.0 / L1
                )
                u2p = spool.tile([128, TILE], bf16, tag="u2", name=f"u2_{pr}")
                nc.vector.scalar_tensor_tensor(
                    u2p, psB1, S1, t2, AOP.mult, AOP.add
                )
                if mode == "b1":
                    qd2 = pr // 2
                    nc.scalar.dma_start(
                        out=outt[:, qd2 * TILE : qd2 * TILE + TILE],
                        in_=u2p[0:4, :],
                    )
                    continue
                sq2 = spool.tile([128, TILE], bf16, tag="sq2", name=f"sq2_{pr}")
                if gp:
                    nc.gpsimd.tensor_tensor(sq2, u2p, u2p, AOP.mult)
                elif sqdve:
                    nc.vector.scalar_tensor_tensor(
                        sq2, u2p, 2.0 * K1, u2p, AOP.add, AOP.mult
                    )
                else:
                    nc.scalar.activation(sq2, u2p, Square, bias=sqb2,
                                         scale=1.0)
                u2ps[qj] = u2p
                sq2s[qj] = sq2

                if qj == 1:
                    # layer 2: block-diag MMs for both pairs, adjacent duos
                    psA2 = pL2.tile([128, TILE], f32, tag="A2",
                                    name=f"A2_{pr}")
                    psB2 = pL2.tile([128, TILE], f32, tag="B2",
                                    name=f"B2_{pr}")
                    for g in range(2):
                        nc.tensor.matmul(
                            psA2[64 * g : 64 * g + 64, :], w2bd, u2ps[g],
                            tile_position=(0, 64 * g),
                            skip_group_check=True,
                        )
                    for g in range(2):
                        nc.tensor.matmul(
                            psB2[64 * g : 64 * g + 64, :], w2bdn, sq2s[g],
                            tile_position=(0, 64 * g),
                            skip_group_check=True,
                            start=True, stop=not gp,
                        )
                    if gp:
                        for g in range(2):
                            nc.tensor.matmul(
                                psB2[64 * g : 64 * g + 64, :], w2bdk,
                                u2ps[g],
                                tile_position=(0, 64 * g),
                                skip_group_check=True,
                                start=False, stop=True,
                            )
                    # layer-2 combine for the quad
                    tq2 = spool.tile([128, TILE], f32, tag="tq2",
                                     name=f"tq2_{pr}")
                    nc.scalar.activation(
                        tq2, psA2, Square, bias=bias2, scale=2.0 / L2
                    )
                    u3p = spool.tile([128, TILE], bf16, tag="u3",
                                     name=f"u3_{pr}")
                    nc.vector.scalar_tensor_tensor(
                        u3p, psB2, S2, tq2, AOP.mult, AOP.add
                    )
                    sq3 = spool.tile([128, TILE], bf16, tag="sq3",
                                     name=f"sq3_{pr}")
                    if gp:
                        nc.gpsimd.tensor_tensor(sq3, u3p, u3p, AOP.mult)
                    elif sqdve:
                        nc.vector.scalar_tensor_tensor(
                            sq3, u3p, 2.0 * K2G, u3p, AOP.add, AOP.mult
                        )
                    else:
                        nc.scalar.activation(
                            sq3, u3p, Square, bias=sqb3, scale=1.0
                        )
                    # layer 3
                    psA3 = pL3.tile([16, TILE], f32, tag="A3", name=f"A3_{pr}")
                    psB3 = pL3.tile([16, TILE], f32, tag="B3", name=f"B3_{pr}")
                    nc.tensor.matmul(psA3, w3bd, u3p)
                    nc.tensor.matmul(psB3, w3bdn, sq3, start=True, stop=False)
                    if gp:
                        nc.tensor.matmul(psB3, w3bdk, u3p,
                                         start=False, stop=False)
                    nc.tensor.matmul(psB3, kf_w, ones_row,
                                     start=False, stop=True)
                    t3 = spool.tile([16, TILE], f32, tag="t3", name=f"t3_{pr}")
                    nc.scalar.activation(
                        t3, psA3, Square, bias=bias3[0:16, :], scale=2.0 / L3
                    )
                    outq = spool.tile([16, TILE], f32, tag="outq",
                                      name=f"outq_{pr}")
                    nc.vector.scalar_tensor_tensor(
                        outq, psB3, S3, t3, AOP.mult, AOP.add
                    )
                    qd = pr // 2
                    dma_eng = nc.sync if outsync else nc.scalar
                    for tq in range(4):
                        tt = 4 * qd + tq
                        dma_eng.dma_start(
                            out=outt[:, tt * TILE : (tt + 1) * TILE],
                            in_=outq[4 * tq : 4 * tq + 4, :],
                        )

    nc.compile()
    return nc


def bench(x, w1, w2, w3, iters=12, reps=1, cores=NCORES, **opt):
    """Device-exec timing: persistent jit + device-resident inputs."""
    import time

    import jax
    from jax.sharding import Mesh, NamedSharding, PartitionSpec
    from jax.experimental.shard_map import shard_map

    import concourse.mybir as mybir
    from concourse import bass2jax
    from concourse.bass2jax import _bass_exec_p, install_neuronx_cc_hook

    nc = _get_nc(reps, **opt)
    install_neuronx_cc_hook()
    in_maps = _host_prep(x, w1, w2, w3)

    partition_name = (
        nc.partition_id_tensor.name if nc.partition_id_tensor else None
    )
    in_names, out_names, out_avals, zero_outs = [], [], [], []
    for alloc in nc.m.functions[0].allocations:
        if not isinstance(alloc, mybir.MemoryLocationSet):
            continue
        name = alloc.memorylocations[0].name
        if alloc.kind == "ExternalInput":
            if name != partition_name:
                in_names.append(name)
        elif alloc.kind == "ExternalOutput":
            out_names.append(name)
            shape = tuple(alloc.tensor_shape)
            dtype = mybir.dt.np(alloc.dtype)
            out_avals.append(jax.core.ShapedArray(shape, dtype))
            zero_outs.append(np.zeros(shape, dtype))
    n_params = len(in_names)
    in_names = in_names + out_names
    if partition_name is not None:
        in_names = in_names + [partition_name]

    def _body(*args):
        operands = list(args)
        if partition_name is not None:
            operands.append(bass2jax.partition_id_tensor())
        outs = _bass_exec_p.bind(
            *operands,
            out_avals=tuple(out_avals),
            in_names=tuple(in_names),
            out_names=tuple(out_names),
            lowering_input_output_aliases=(),
            sim_require_finite=True,
            sim_require_nnan=True,
            nc=nc,
        )
        return tuple(outs)

    devices = jax.devices()[:cores]
    mesh = Mesh(np.asarray(devices), ("core",))
    in_specs = (PartitionSpec("core"),) * (n_params + len(out_names))
    out_specs = (PartitionSpec("core"),) * len(out_names)
    fn = jax.jit(
        shard_map(_body, mesh=mesh, in_specs=in_specs, out_specs=out_specs,
                  check_rep=False),
        keep_unused=True,
    )
    sh = NamedSharding(mesh, PartitionSpec("core"))
    dev_in = [
        jax.device_put(
            np.concatenate([in_maps[c][nm] for c in range(cores)], axis=0), sh
        )
        for nm in in_names[:n_params]
    ]
    dev_zero = [
        jax.device_put(
            np.zeros((cores * z.shape[0], *z.shape[1:]), z.dtype), sh
        )
        for z in zero_outs
    ]
    out = fn(*dev_in, *dev_zero)
    jax.block_until_ready(out)
    times = []
    for _ in range(iters):
        t0 = time.perf_counter()
        out = fn(*dev_in, *dev_zero)
        jax.block_until_ready(out)
        times.append(time.perf_counter() - t0)
    out_np = np.asarray(out[0]).reshape(cores, *out_avals[0].shape)
    result = np.concatenate([out_np[c].T for c in range(cores)], axis=0)
    return result.astype(np.float32), times
